# revision 1
# baseline (speedup 1.0000x reference)
"""GraphSAGE layer on 8 Trainium2 NeuronCores.

Strategy (1D graph partitioning):
  - Nodes (output rows / destination segments) sharded across 8 cores,
    6250 per core.  Edges are bucketed to the core owning their
    destination and sorted by destination; the full `features` table is
    replicated in DRAM on every core.
  - Source rows are fetched with `dma_gather` (custom SWDGE gather,
    one descriptor per edge).  Indices are int16, so the feature table
    is split at row 32768: per destination tile the edge list is
    [A-half | B-half], each padded to a multiple of 128 with
    (idx=0, w=0) dummy edges, uniform across cores so the SPMD program
    is identical.
  - Per 128-destination tile the kernel accumulates
        neighborT[64, 128] += G_block.T @ S_block
    in PSUM, where G_block [128e, 64] are gathered source rows and
    S_block [128e, 128] is a one-hot(dst_rel)*weight matrix built on
    DVE with one fused tensor_scalar (is_equal, mult) per block.
  - Self features arrive pre-transposed from the host; the final
    linear layer is one [128,128]x[128,64] matmul per tile, then bias
    add and row L2-normalization.
"""

import sys

if "/opt/trn_rl_repo" not in sys.path:
    sys.path.insert(0, "/opt/trn_rl_repo")

import numpy as np

import concourse.bacc as bacc
import concourse.bass as bass
import concourse.tile as tile
from concourse import mybir
from concourse.bass_utils import run_bass_kernel_spmd

N_NODES = 50000
N_EDGES = 800000
D = 64
C = 8
NPC = N_NODES // C  # 6250
P = 128
T = (NPC + P - 1) // P  # 49 dst tiles/core
LAST_ROWS = NPC - (T - 1) * P  # 106
SPLIT = 32768  # int16 index limit for dma_gather
BLKS_PER_CALL = 8  # 1024 indices per dma_gather (1920 crashes the runtime)

_last_results = None


def _prep(edge_src, edge_dst, edge_weight):
    """Returns per-core [128, TB] edge-scalar layouts plus the uniform
    block structure (nbA[t], nbB[t] block counts per dst tile)."""
    order = np.argsort(edge_dst, kind="stable")
    src_s = edge_src[order].astype(np.int64)
    dst_s = edge_dst[order].astype(np.int64)
    w_s = edge_weight[order].astype(np.float32)

    cid = dst_s // NPC
    loc = dst_s - cid * NPC
    tid = loc // P
    half = (src_s >= SPLIT).astype(np.int64)  # 0=A, 1=B
    # sort key: (core, tile, half) — stable keeps dst order within
    key = (cid * T + tid) * 2 + half
    order2 = np.argsort(key, kind="stable")
    src_s, dst_s, w_s, cid, loc, tid, half, key = (
        a[order2] for a in (src_s, dst_s, w_s, cid, loc, tid, half, key)
    )

    counts = np.bincount(key, minlength=C * T * 2).reshape(C, T, 2)
    nidxA = np.maximum(1, counts[:, :, 0].max(axis=0))  # [T] exact gather counts
    nidxB = counts[:, :, 1].max(axis=0)  # [T] may be 0
    nbA = (nidxA + P - 1) // P
    nbB = (nidxB + P - 1) // P
    nbt = nbA + nbB
    b0 = np.concatenate([[0], np.cumsum(nbt)])  # tile block offsets
    tb = int(b0[-1])

    # position of each edge inside its (core,tile,half) segment
    seg_starts = np.concatenate([[0], np.cumsum(counts.reshape(-1))])
    j = np.arange(len(dst_s)) - np.repeat(seg_starts[:-1], counts.reshape(-1))
    # block column (global within [128, TB] layout)
    half_off = np.where(half == 1, nbA[tid], 0)
    col = b0[tid] + half_off + j // P
    part = j % P

    dstrel = np.zeros((C, P, tb), np.float32)
    wv = np.zeros((C, P, tb), np.float32)
    dstrel[cid, part, col] = (loc - tid * P).astype(np.float32)
    wv[cid, part, col] = w_s

    # wrapped int16 index array [16, TB*8] replicated to 128 partitions.
    # within each (tile, half) segment the list restarts at the segment's
    # block boundary, so e%16 / e//16 wrapping is slice-consistent.
    idxw = np.zeros((C, 16, tb * 8), np.int16)
    e_in_list = (half_off * 0 + j)  # position within half-list
    wcol = (b0[tid] + half_off) * 8 + e_in_list // 16
    wrow = e_in_list % 16
    idxw[cid, wrow, wcol] = (src_s - half * SPLIT).astype(np.int16)

    nbA = [int(x) for x in nbA]
    nbB = [int(x) for x in nbB]
    nidxA = [int(x) for x in nidxA]
    nidxB = [int(x) for x in nidxB]
    b0 = [int(x) for x in b0]
    return dstrel, wv, idxw, nbA, nbB, nidxA, nidxB, b0, tb


def _build(nbA, nbB, nidxA, nidxB, b0, tb):
    nc = bacc.Bacc()
    f32 = mybir.dt.float32

    feat = nc.declare_dram_parameter("feat", [N_NODES, D], f32, isOutput=False)
    idxw = nc.declare_dram_parameter("idxw", [P, tb * 8], mybir.dt.int16, isOutput=False)
    dstrel = nc.declare_dram_parameter("dstrel", [P, tb], f32, isOutput=False)
    wv = nc.declare_dram_parameter("wv", [P, tb], f32, isOutput=False)
    featT = nc.declare_dram_parameter("featT", [D, T * P], f32, isOutput=False)
    wt = nc.declare_dram_parameter("wt", [2 * D, D], f32, isOutput=False)
    biasb = nc.declare_dram_parameter("biasb", [P, D], f32, isOutput=False)
    iota = nc.declare_dram_parameter("iota", [P, P], f32, isOutput=False)
    out = nc.declare_dram_parameter("out", [NPC, D], f32, isOutput=True)

    nbmax = max(a + b for a, b in zip(nbA, nbB))

    with tile.TileContext(nc) as tc:
        with (
            tc.tile_pool(name="singles", bufs=1) as singles,
            tc.tile_pool(name="gpool", bufs=3) as gpool,
            tc.tile_pool(name="spool", bufs=6) as spool,
            tc.tile_pool(name="cpool", bufs=3) as cpool,
            tc.tile_pool(name="opool", bufs=3) as opool,
            tc.tile_pool(name="stat", bufs=6) as stat,
            tc.tile_pool(name="pnT", bufs=2, space="PSUM") as pnT,
            tc.tile_pool(name="pout", bufs=2, space="PSUM") as pout,
        ):
            idx_sb = singles.tile([P, tb * 8], mybir.dt.int16)
            dstrel_sb = singles.tile([P, tb], f32)
            wv_sb = singles.tile([P, tb], f32)
            wt_sb = singles.tile([2 * D, D], f32)
            bias_sb = singles.tile([P, D], f32)
            iota_sb = singles.tile([P, P], f32)
            nc.sync.dma_start(out=idx_sb[:], in_=idxw[:])
            nc.sync.dma_start(out=dstrel_sb[:], in_=dstrel[:])
            nc.sync.dma_start(out=wv_sb[:], in_=wv[:])
            nc.sync.dma_start(out=wt_sb[:], in_=wt[:])
            nc.sync.dma_start(out=bias_sb[:], in_=biasb[:])
            nc.sync.dma_start(out=iota_sb[:], in_=iota[:])

            for t in range(T):
                ka, kb = nbA[t], nbB[t]
                g = gpool.tile([P, nbmax * D], f32, tag="g")
                # gather calls, <= BLKS_PER_CALL blocks each; last call of a
                # half uses the exact (max-over-cores) index count — trailing
                # slots of the last block stay stale and are zeroed by S (w=0)
                for base_tbl, nblk, nidx_half, coff in (
                    (0, ka, nidxA[t], 0),
                    (SPLIT, kb, nidxB[t], ka),
                ):
                    k0 = 0
                    while k0 < nblk:
                        k1 = min(k0 + BLKS_PER_CALL, nblk)
                        nidx = min((k1 - k0) * P, nidx_half - k0 * P)
                        gcol = (coff + k0) * D
                        icol = (b0[t] + coff + k0) * 8
                        nc.gpsimd.dma_gather(
                            out_ap=g[:, gcol : gcol + (k1 - k0) * D].rearrange(
                                "p (n e) -> p n e", e=D
                            ),
                            in_ap=feat[base_tbl:, :],
                            idxs_ap=idx_sb[:, icol : icol + (nidx + 15) // 16],
                            num_idxs=nidx,
                            num_idxs_reg=nidx,
                            elem_size=D,
                        )
                        k0 = k1
                nt = pnT.tile([D, P], f32)
                for i in range(ka + kb):
                    col = b0[t] + i
                    s = spool.tile([P, P], f32, tag="s")
                    nc.vector.tensor_scalar(
                        out=s[:],
                        in0=iota_sb[:],
                        scalar1=dstrel_sb[:, col : col + 1],
                        scalar2=wv_sb[:, col : col + 1],
                        op0=mybir.AluOpType.is_equal,
                        op1=mybir.AluOpType.mult,
                    )
                    nc.tensor.matmul(
                        out=nt[:],
                        lhsT=g[:, i * D : (i + 1) * D],
                        rhs=s[:],
                        start=(i == 0),
                        stop=(i == ka + kb - 1),
                    )
                comb = cpool.tile([P, P], f32, tag="comb")
                nc.sync.dma_start(out=comb[:D, :], in_=featT[:, t * P : (t + 1) * P])
                nc.vector.tensor_copy(out=comb[D:, :], in_=nt[:])
                po = pout.tile([P, D], f32)
                nc.tensor.matmul(
                    out=po[:], lhsT=comb[:], rhs=wt_sb[:], start=True, stop=True
                )
                o = opool.tile([P, D], f32, tag="o")
                nc.vector.tensor_add(out=o[:], in0=po[:], in1=bias_sb[:])
                sq = opool.tile([P, D], f32, tag="sq")
                ssum = stat.tile([P, 1], f32, tag="ssum")
                nc.scalar.activation(
                    out=sq[:],
                    in_=o[:],
                    func=mybir.ActivationFunctionType.Square,
                    accum_out=ssum[:],
                )
                nrm = stat.tile([P, 1], f32, tag="nrm")
                nc.scalar.activation(
                    out=nrm[:], in_=ssum[:], func=mybir.ActivationFunctionType.Sqrt
                )
                nc.vector.tensor_scalar_max(out=nrm[:], in0=nrm[:], scalar1=1e-12)
                rin = stat.tile([P, 1], f32, tag="rin")
                nc.vector.reciprocal(out=rin[:], in_=nrm[:])
                nc.vector.tensor_scalar_mul(out=o[:], in0=o[:], scalar1=rin[:])
                rows = LAST_ROWS if t == T - 1 else P
                nc.sync.dma_start(out=out[t * P : t * P + rows, :], in_=o[:rows, :])

    nc.compile()
    return nc


def kernel(features, edge_src, edge_dst, edge_weight, W, b, _cache={}):
    global _last_results
    features = np.ascontiguousarray(features, dtype=np.float32)
    edge_src = np.ascontiguousarray(edge_src, dtype=np.int32)
    edge_dst = np.ascontiguousarray(edge_dst, dtype=np.int32)
    edge_weight = np.ascontiguousarray(edge_weight, dtype=np.float32)
    W = np.ascontiguousarray(W, dtype=np.float32)
    b = np.ascontiguousarray(b, dtype=np.float32)

    dstrel, wv, idxw, nbA, nbB, nidxA, nidxB, b0, tb = _prep(
        edge_src, edge_dst, edge_weight
    )

    featT = features.T
    featT_pad = np.zeros((C, D, T * P), np.float32)
    for c in range(C):
        featT_pad[c, :, :NPC] = featT[:, c * NPC : (c + 1) * NPC]
    wt = np.ascontiguousarray(W.T)
    biasb = np.ascontiguousarray(np.broadcast_to(b, (P, D)))
    iota = np.tile(np.arange(P, dtype=np.float32), (P, 1))

    key = ("k3", tb, tuple(nidxA), tuple(nidxB))
    if key not in _cache:
        _cache.clear()
        _cache[key] = _build(nbA, nbB, nidxA, nidxB, b0, tb)
    nc = _cache[key]

    in_maps = [
        {
            "feat": features,
            "idxw": np.ascontiguousarray(np.tile(idxw[c], (8, 1))),
            "dstrel": np.ascontiguousarray(dstrel[c]),
            "wv": np.ascontiguousarray(wv[c]),
            "featT": featT_pad[c],
            "wt": wt,
            "biasb": biasb,
            "iota": iota,
        }
        for c in range(C)
    ]
    import os

    trace = bool(os.environ.get("GS_TRACE"))
    res = run_bass_kernel_spmd(
        nc, in_maps, core_ids=list(range(C)), trace=trace
    )
    _last_results = res
    out = np.concatenate([res.results[c]["out"] for c in range(C)], axis=0)
    return out.astype(np.float32)



# revision 5
# speedup vs baseline: 1.8067x; 1.8067x over previous
"""GraphSAGE layer on 8 Trainium2 NeuronCores.

Strategy (1D graph partitioning):
  - Nodes (output rows / destination segments) sharded across 8 cores,
    6250 per core.  Edges are bucketed to the core owning their
    destination and sorted by destination; the full feature table is
    replicated in DRAM on every core as fp16 with each row duplicated
    ([50000, 128]) so a gather element is 256 bytes (SWDGE minimum).
  - Source rows are fetched with `dma_gather` (SWDGE) spread across 4
    SWDGE queues (each queue runs on its own Q7 cpu pair, so 4 queues
    generate descriptors concurrently).  Indices are int16, so the
    table is split at row 32768; per destination tile the edge list is
    [A-half | B-half], padded with -1 (trailing -1s are trimmed by the
    ucode, so padding costs no descriptors), uniform across cores so
    the SPMD program is identical.
  - Per 128-destination tile the kernel accumulates
        neighborT[64, 128] += G_block.T @ S_block
    in PSUM, where G_block [128e, 64] are gathered fp16 source rows and
    S_block [128e, 128] is a one-hot(dst_rel)*weight matrix built on
    DVE in fp16 with two batched tensor_tensor ops per tile
    (is_equal against a tiled iota, then multiply by broadcast w).
  - Self features arrive pre-transposed (fp16); the final linear layer
    is one fp16 [128,128]x[128,64] matmul per tile, then f32 bias add
    and row L2-normalization.
"""

import sys

if "/opt/trn_rl_repo" not in sys.path:
    sys.path.insert(0, "/opt/trn_rl_repo")

import numpy as np

import concourse.bacc as bacc
import concourse.bass as bass
import concourse.tile as tile
from concourse import mybir
from concourse.bass_utils import run_bass_kernel_spmd

N_NODES = 50000
N_EDGES = 800000
D = 64
C = 8
NPC = N_NODES // C  # 6250
P = 128
T = (NPC + P - 1) // P  # 49 dst tiles/core
LAST_ROWS = NPC - (T - 1) * P  # 106
SPLIT = 32768  # int16 index limit for dma_gather
BLKS_PER_CALL = 8  # 1024 indices per dma_gather (2048 crashes the runtime)
NQUEUES = 4  # SWDGE queues (one Q7 cpu pair each)

_last_results = None


def _prep(edge_src, edge_dst, edge_weight):
    """Returns per-core [128, TB] edge-scalar layouts plus the uniform
    block structure (nbA[t], nbB[t] block counts per dst tile)."""
    order = np.argsort(edge_dst, kind="stable")
    src_s = edge_src[order].astype(np.int64)
    dst_s = edge_dst[order].astype(np.int64)
    w_s = edge_weight[order].astype(np.float32)

    cid = dst_s // NPC
    loc = dst_s - cid * NPC
    tid = loc // P
    half = (src_s >= SPLIT).astype(np.int64)  # 0=A, 1=B
    # sort key: (core, tile, half) — stable keeps dst order within
    key = (cid * T + tid) * 2 + half
    order2 = np.argsort(key, kind="stable")
    src_s, dst_s, w_s, cid, loc, tid, half, key = (
        a[order2] for a in (src_s, dst_s, w_s, cid, loc, tid, half, key)
    )

    counts = np.bincount(key, minlength=C * T * 2).reshape(C, T, 2)
    nidxA = np.maximum(1, counts[:, :, 0].max(axis=0))  # [T] max gather counts
    nidxB = counts[:, :, 1].max(axis=0)  # [T] may be 0
    nbA = (nidxA + P - 1) // P
    nbB = (nidxB + P - 1) // P
    nbt = nbA + nbB
    b0 = np.concatenate([[0], np.cumsum(nbt)])  # tile block offsets
    tb = int(b0[-1])

    # position of each edge inside its (core,tile,half) segment
    seg_starts = np.concatenate([[0], np.cumsum(counts.reshape(-1))])
    j = np.arange(len(dst_s)) - np.repeat(seg_starts[:-1], counts.reshape(-1))
    # block column (global within [128, TB] layout)
    half_off = np.where(half == 1, nbA[tid], 0)
    col = b0[tid] + half_off + j // P
    part = j % P

    dstrel = np.zeros((C, P, tb), np.float16)
    wv = np.zeros((C, P, tb), np.float16)
    dstrel[cid, part, col] = (loc - tid * P).astype(np.float16)
    wv[cid, part, col] = w_s.astype(np.float16)

    # wrapped int16 index array [16, TB*8] replicated to 128 partitions.
    # within each (tile, half) segment the list restarts at the segment's
    # block boundary, so e%16 / e//16 wrapping is slice-consistent.
    # pad = 0 (w=0 kills the dummy contributions in S).
    idxw = np.zeros((C, 16, tb * 8), np.int16)
    wcol = (b0[tid] + half_off) * 8 + j // 16
    wrow = j % 16
    idxw[cid, wrow, wcol] = (src_s - half * SPLIT).astype(np.int16)

    nbA = [int(x) for x in nbA]
    nbB = [int(x) for x in nbB]
    nidxA = [int(x) for x in nidxA]
    nidxB = [int(x) for x in nidxB]
    b0 = [int(x) for x in b0]
    return dstrel, wv, idxw, nbA, nbB, nidxA, nidxB, b0, tb


def _build(nbA, nbB, nidxA, nidxB, b0, tb):
    nc = bacc.Bacc(num_swdge_queues=NQUEUES)
    f32 = mybir.dt.float32
    f16 = mybir.dt.float16

    feat2 = nc.declare_dram_parameter("feat2", [N_NODES, 2 * D], f16, isOutput=False)
    idxw = nc.declare_dram_parameter("idxw", [P, tb * 8], mybir.dt.int16, isOutput=False)
    dstrel = nc.declare_dram_parameter("dstrel", [P, tb], f16, isOutput=False)
    wv = nc.declare_dram_parameter("wv", [P, tb], f16, isOutput=False)
    featT = nc.declare_dram_parameter("featT", [D, T * P], f16, isOutput=False)
    wt = nc.declare_dram_parameter("wt", [2 * D, D], f16, isOutput=False)
    biasb = nc.declare_dram_parameter("biasb", [P, D], f32, isOutput=False)
    iota = nc.declare_dram_parameter("iota", [P, P], f16, isOutput=False)
    out = nc.declare_dram_parameter("out", [NPC, D], f32, isOutput=True)

    nbmax = max(a + b for a, b in zip(nbA, nbB))
    E = 2 * D  # gather element: duplicated fp16 row = 256 bytes

    with tile.TileContext(nc) as tc:
        with (
            tc.tile_pool(name="singles", bufs=1) as singles,
            tc.tile_pool(name="gpool", bufs=4) as gpool,
            tc.tile_pool(name="spool", bufs=3) as spool,
            tc.tile_pool(name="cpool", bufs=3) as cpool,
            tc.tile_pool(name="opool", bufs=3) as opool,
            tc.tile_pool(name="stat", bufs=6) as stat,
            tc.tile_pool(name="pnT", bufs=2, space="PSUM") as pnT,
            tc.tile_pool(name="pout", bufs=2, space="PSUM") as pout,
        ):
            idx_sb = singles.tile([P, tb * 8], mybir.dt.int16)
            dstrel_sb = singles.tile([P, tb], f16)
            wv_sb = singles.tile([P, tb], f16)
            wt_sb = singles.tile([2 * D, D], f16)
            bias_sb = singles.tile([P, D], f32)
            iota_sb = singles.tile([P, nbmax * P], f16)
            nc.sync.dma_start(out=idx_sb[:], in_=idxw[:])
            nc.sync.dma_start(out=dstrel_sb[:], in_=dstrel[:])
            nc.sync.dma_start(out=wv_sb[:], in_=wv[:])
            nc.sync.dma_start(out=wt_sb[:], in_=wt[:])
            nc.sync.dma_start(out=bias_sb[:], in_=biasb[:])
            # iota tiled nbmax times across the free dim
            for i in range(nbmax):
                nc.sync.dma_start(out=iota_sb[:, i * P : (i + 1) * P], in_=iota[:])

            qn = 0
            for t in range(T):
                ka, kb = nbA[t], nbB[t]
                nb = ka + kb
                g = gpool.tile([P, nbmax * E], f16, tag="g")
                if t < 4:
                    # stale-SBUF guard: ungathered slots are killed by w=0
                    # in S, but initial SBUF garbage could be NaN and
                    # NaN*0 stays NaN — zero the first round of buffers.
                    nc.vector.memset(g[:], 0.0)
                # gather calls, <= BLKS_PER_CALL blocks each, queues
                # round-robin so all four Q7 pairs generate descriptors
                for base_tbl, nblk, nidx_half, coff in (
                    (0, ka, nidxA[t], 0),
                    (SPLIT, kb, nidxB[t], ka),
                ):
                    k0 = 0
                    while k0 < nblk:
                        k1 = min(k0 + BLKS_PER_CALL, nblk)
                        nidx = min((k1 - k0) * P, nidx_half - k0 * P)
                        gcol = (coff + k0) * E
                        icol = (b0[t] + coff + k0) * 8
                        nc.gpsimd.dma_gather(
                            out_ap=g[:, gcol : gcol + (k1 - k0) * E].rearrange(
                                "p (n e) -> p n e", e=E
                            ),
                            in_ap=feat2[base_tbl:, :],
                            idxs_ap=idx_sb[:, icol : icol + (nidx + 15) // 16],
                            num_idxs=nidx,
                            num_idxs_reg=nidx,
                            elem_size=E,
                            queue_num=qn % NQUEUES,
                        )
                        qn += 1
                        k0 = k1
                # batched S build: one is_equal + one mult for all nb blocks
                s = spool.tile([P, nbmax * P], f16, tag="s")
                nc.vector.tensor_tensor(
                    out=s[:, : nb * P].rearrange("p (n q) -> p n q", q=P),
                    in0=iota_sb[:, : nb * P].rearrange("p (n q) -> p n q", q=P),
                    in1=dstrel_sb[:, b0[t] : b0[t] + nb].to_broadcast([P, nb, P]),
                    op=mybir.AluOpType.is_equal,
                )
                nc.vector.tensor_tensor(
                    out=s[:, : nb * P].rearrange("p (n q) -> p n q", q=P),
                    in0=s[:, : nb * P].rearrange("p (n q) -> p n q", q=P),
                    in1=wv_sb[:, b0[t] : b0[t] + nb].to_broadcast([P, nb, P]),
                    op=mybir.AluOpType.mult,
                )
                nt = pnT.tile([D, P], f32)
                for i in range(nb):
                    nc.tensor.matmul(
                        out=nt[:],
                        lhsT=g[:, i * E : i * E + D],
                        rhs=s[:, i * P : (i + 1) * P],
                        start=(i == 0),
                        stop=(i == nb - 1),
                    )
                comb = cpool.tile([P, P], f16, tag="comb")
                nc.sync.dma_start(out=comb[:D, :], in_=featT[:, t * P : (t + 1) * P])
                nc.vector.tensor_copy(out=comb[D:, :], in_=nt[:])
                po = pout.tile([P, D], f32)
                nc.tensor.matmul(
                    out=po[:], lhsT=comb[:], rhs=wt_sb[:], start=True, stop=True
                )
                o = opool.tile([P, D], f32, tag="o")
                nc.vector.tensor_add(out=o[:], in0=po[:], in1=bias_sb[:])
                sq = opool.tile([P, D], f32, tag="sq")
                ssum = stat.tile([P, 1], f32, tag="ssum")
                nc.scalar.activation(
                    out=sq[:],
                    in_=o[:],
                    func=mybir.ActivationFunctionType.Square,
                    accum_out=ssum[:],
                )
                nrm = stat.tile([P, 1], f32, tag="nrm")
                nc.scalar.activation(
                    out=nrm[:], in_=ssum[:], func=mybir.ActivationFunctionType.Sqrt
                )
                nc.vector.tensor_scalar_max(out=nrm[:], in0=nrm[:], scalar1=1e-12)
                rin = stat.tile([P, 1], f32, tag="rin")
                nc.vector.reciprocal(out=rin[:], in_=nrm[:])
                nc.vector.tensor_scalar_mul(out=o[:], in0=o[:], scalar1=rin[:])
                rows = LAST_ROWS if t == T - 1 else P
                nc.sync.dma_start(out=out[t * P : t * P + rows, :], in_=o[:rows, :])

    nc.compile()
    return nc


def kernel(features, edge_src, edge_dst, edge_weight, W, b, _cache={}):
    global _last_results
    features = np.ascontiguousarray(features, dtype=np.float32)
    edge_src = np.ascontiguousarray(edge_src, dtype=np.int32)
    edge_dst = np.ascontiguousarray(edge_dst, dtype=np.int32)
    edge_weight = np.ascontiguousarray(edge_weight, dtype=np.float32)
    W = np.ascontiguousarray(W, dtype=np.float32)
    b = np.ascontiguousarray(b, dtype=np.float32)

    dstrel, wv, idxw, nbA, nbB, nidxA, nidxB, b0, tb = _prep(
        edge_src, edge_dst, edge_weight
    )

    f16 = features.astype(np.float16)
    feat2 = np.ascontiguousarray(np.concatenate([f16, f16], axis=1))  # [N, 128]
    featT = features.T.astype(np.float16)
    featT_pad = np.zeros((C, D, T * P), np.float16)
    for c in range(C):
        featT_pad[c, :, :NPC] = featT[:, c * NPC : (c + 1) * NPC]
    wt = np.ascontiguousarray(W.T.astype(np.float16))
    biasb = np.ascontiguousarray(np.broadcast_to(b, (P, D)).astype(np.float32))
    iota = np.tile(np.arange(P, dtype=np.float16), (P, 1))

    key = ("k4", tb, tuple(nidxA), tuple(nidxB))
    if key not in _cache:
        _cache.clear()
        _cache[key] = _build(nbA, nbB, nidxA, nidxB, b0, tb)
    nc = _cache[key]

    in_maps = [
        {
            "feat2": feat2,
            "idxw": np.ascontiguousarray(np.tile(idxw[c], (8, 1))),
            "dstrel": np.ascontiguousarray(dstrel[c]),
            "wv": np.ascontiguousarray(wv[c]),
            "featT": featT_pad[c],
            "wt": wt,
            "biasb": biasb,
            "iota": iota,
        }
        for c in range(C)
    ]
    import os

    trace = bool(os.environ.get("GS_TRACE"))
    res = run_bass_kernel_spmd(
        nc, in_maps, core_ids=list(range(C)), trace=trace
    )
    _last_results = res
    out = np.concatenate([res.results[c]["out"] for c in range(C)], axis=0)
    return out.astype(np.float32)


# revision 7
# speedup vs baseline: 3.1995x; 1.7710x over previous
"""GraphSAGE layer on 8 Trainium2 NeuronCores.

Strategy (1D graph partitioning):
  - Nodes (output rows / destination segments) sharded across 8 cores,
    6250 per core.  Edges are bucketed to the core owning their
    destination and sorted by destination; the full feature table is
    replicated in DRAM on every core as fp16 with each row duplicated
    ([50000, 128]) so a gather element is 256 bytes (SWDGE minimum).
  - Source rows are fetched with `dma_gather` (SWDGE) spread across 4
    SWDGE queues (each queue runs on its own Q7 cpu pair, so 4 queues
    generate descriptors concurrently).  Indices are int16, so the
    table is split at row 32768; per destination tile the edge list is
    [A-half | B-half], padded with (idx=0, w=0), uniform across cores
    so the SPMD program is identical.
  - Per 128-destination tile the kernel accumulates neighborT [64,128]
    in PSUM.  Because edges are dst-sorted, each 128-edge block only
    spans a narrow band of destinations, so the one-hot scatter matrix
    S_block is built on a [128, W_t] strip (W_t = per-tile max band,
    static across cores) with two batched broadcast tensor_tensor ops
    per tile, and each block's matmul writes the PSUM column slice
    [a_i, a_i+W_t).  A zeroing matmul (start=True) and a closing
    (stop=True) matmul bracket the strips.
  - Self features arrive pre-transposed (fp16); the final linear layer
    is one fp16 [128,128]x[128,64] matmul per tile, then f32 bias add
    and row L2-normalization (Square/Sqrt + copy-scale on the Scalar
    engine, max/reciprocal on DVE).
"""

import sys

if "/opt/trn_rl_repo" not in sys.path:
    sys.path.insert(0, "/opt/trn_rl_repo")

import numpy as np

import concourse.bacc as bacc
import concourse.bass as bass
import concourse.tile as tile
from concourse import mybir
from concourse.bass_utils import run_bass_kernel_spmd

N_NODES = 50000
N_EDGES = 800000
D = 64
C = 8
NPC = N_NODES // C  # 6250
P = 128
T = (NPC + P - 1) // P  # 49 dst tiles/core
LAST_ROWS = NPC - (T - 1) * P  # 106
SPLIT = 32768  # int16 index limit for dma_gather
BLKS_PER_CALL = 8  # 1024 indices per dma_gather (2048 overflows the ring)
NQUEUES = 4  # SWDGE queues (one Q7 cpu pair each)

_last_results = None


def _prep(edge_src, edge_dst, edge_weight):
    """Returns per-core [128, TB] edge-scalar layouts plus the uniform
    block structure (nbA[t], nbB[t], strip offsets/widths per tile)."""
    order = np.argsort(edge_dst, kind="stable")
    src_s = edge_src[order].astype(np.int64)
    dst_s = edge_dst[order].astype(np.int64)
    w_s = edge_weight[order].astype(np.float32)

    cid = dst_s // NPC
    loc = dst_s - cid * NPC
    tid = loc // P
    half = (src_s >= SPLIT).astype(np.int64)  # 0=A, 1=B
    # sort key: (core, tile, half) — stable keeps dst order within
    key = (cid * T + tid) * 2 + half
    order2 = np.argsort(key, kind="stable")
    src_s, dst_s, w_s, cid, loc, tid, half, key = (
        a[order2] for a in (src_s, dst_s, w_s, cid, loc, tid, half, key)
    )

    counts = np.bincount(key, minlength=C * T * 2).reshape(C, T, 2)
    nidxA = np.maximum(1, counts[:, :, 0].max(axis=0))  # [T] max gather counts
    nidxB = counts[:, :, 1].max(axis=0)  # [T] may be 0
    nbA = (nidxA + P - 1) // P
    nbB = (nidxB + P - 1) // P
    nbt = nbA + nbB
    b0 = np.concatenate([[0], np.cumsum(nbt)])  # tile block offsets
    tb = int(b0[-1])

    # position of each edge inside its (core,tile,half) segment
    seg_starts = np.concatenate([[0], np.cumsum(counts.reshape(-1))])
    j = np.arange(len(dst_s)) - np.repeat(seg_starts[:-1], counts.reshape(-1))
    # block column (global within [128, TB] layout)
    half_off = np.where(half == 1, nbA[tid], 0)
    col = b0[tid] + half_off + j // P
    part = j % P
    drel = (loc - tid * P).astype(np.int64)  # 0..127 within tile

    # per-(tile, block) dst band across all cores -> static strip
    lo = np.full(tb, 128, np.int64)
    hi = np.full(tb, -1, np.int64)
    np.minimum.at(lo, col, drel)
    np.maximum.at(hi, col, drel)
    lo = np.minimum(lo, 127)
    hi = np.maximum(hi, lo)
    Wt = []  # strip width per tile (multiple of 16)
    a_blk = np.zeros(tb, np.int64)  # strip start per block
    for t in range(T):
        blks = slice(b0[t], b0[t + 1])
        span = int((hi[blks] - lo[blks]).max()) + 1
        w = min(128, ((span + 15) // 16) * 16)
        Wt.append(w)
        a_blk[blks] = np.minimum(lo[blks], 128 - w)
    a_blk_t = a_blk[col]

    dstrel = np.full((C, P, tb), -1.0, np.float16)  # pad: never matches iota
    wv = np.zeros((C, P, tb), np.float16)
    dstrel[cid, part, col] = (drel - a_blk_t).astype(np.float16)
    wv[cid, part, col] = w_s.astype(np.float16)

    # wrapped int16 index array [16, TB*8] replicated to 128 partitions.
    # within each (tile, half) segment the list restarts at the segment's
    # block boundary, so e%16 / e//16 wrapping is slice-consistent.
    # pad = 0 (w=0 kills the dummy contributions in S).
    idxw = np.zeros((C, 16, tb * 8), np.int16)
    wcol = (b0[tid] + half_off) * 8 + j // 16
    wrow = j % 16
    idxw[cid, wrow, wcol] = (src_s - half * SPLIT).astype(np.int16)

    nbA = [int(x) for x in nbA]
    nbB = [int(x) for x in nbB]
    nidxA = [int(x) for x in nidxA]
    nidxB = [int(x) for x in nidxB]
    b0 = [int(x) for x in b0]
    a_blk = [int(x) for x in a_blk]
    return dstrel, wv, idxw, nbA, nbB, nidxA, nidxB, b0, tb, Wt, a_blk


def _build(nbA, nbB, nidxA, nidxB, b0, tb, Wt, a_blk):
    nc = bacc.Bacc(num_swdge_queues=NQUEUES)
    f32 = mybir.dt.float32
    f16 = mybir.dt.float16

    feat2 = nc.declare_dram_parameter("feat2", [N_NODES, 2 * D], f16, isOutput=False)
    idxw = nc.declare_dram_parameter("idxw", [P, tb * 8], mybir.dt.int16, isOutput=False)
    dstrel = nc.declare_dram_parameter("dstrel", [P, tb], f16, isOutput=False)
    wv = nc.declare_dram_parameter("wv", [P, tb], f16, isOutput=False)
    featT = nc.declare_dram_parameter("featT", [D, T * P], f16, isOutput=False)
    wt = nc.declare_dram_parameter("wt", [2 * D, D], f16, isOutput=False)
    biasb = nc.declare_dram_parameter("biasb", [P, D], f32, isOutput=False)
    iota = nc.declare_dram_parameter("iota", [P, P], f16, isOutput=False)
    out = nc.declare_dram_parameter("out", [NPC, D], f32, isOutput=True)

    nbmax = max(a + b for a, b in zip(nbA, nbB))
    swmax = max(
        (nbA[t] + nbB[t]) * Wt[t] for t in range(T)
    )  # S strip tile columns
    E = 2 * D  # gather element: duplicated fp16 row = 256 bytes

    with tile.TileContext(nc) as tc:
        with (
            tc.tile_pool(name="singles", bufs=1) as singles,
            tc.tile_pool(name="gpool", bufs=6) as gpool,
            tc.tile_pool(name="spool", bufs=3) as spool,
            tc.tile_pool(name="cpool", bufs=3) as cpool,
            tc.tile_pool(name="opool", bufs=3) as opool,
            tc.tile_pool(name="stat", bufs=6) as stat,
            tc.tile_pool(name="pnT", bufs=2, space="PSUM") as pnT,
            tc.tile_pool(name="pout", bufs=2, space="PSUM") as pout,
        ):
            idx_sb = singles.tile([P, tb * 8], mybir.dt.int16)
            dstrel_sb = singles.tile([P, tb], f16)
            wv_sb = singles.tile([P, tb], f16)
            wt_sb = singles.tile([2 * D, D], f16)
            bias_sb = singles.tile([P, D], f32)
            iota_sb = singles.tile([P, P], f16)
            zeros_sb = singles.tile([P, P], f16)
            nc.sync.dma_start(out=idx_sb[:], in_=idxw[:])
            nc.sync.dma_start(out=dstrel_sb[:], in_=dstrel[:])
            nc.sync.dma_start(out=wv_sb[:], in_=wv[:])
            nc.sync.dma_start(out=wt_sb[:], in_=wt[:])
            nc.sync.dma_start(out=bias_sb[:], in_=biasb[:])
            nc.sync.dma_start(out=iota_sb[:], in_=iota[:])
            nc.vector.memset(zeros_sb[:], 0.0)

            qn = 0
            for t in range(T):
                ka, kb = nbA[t], nbB[t]
                nb = ka + kb
                W = Wt[t]
                g = gpool.tile([P, nbmax * E], f16, tag="g")
                if t < 6:
                    # stale-SBUF guard: ungathered slots are killed by w=0
                    # in S, but initial SBUF garbage could be NaN and
                    # NaN*0 stays NaN — zero the first round of buffers.
                    nc.vector.memset(g[:], 0.0)
                # gather calls, <= BLKS_PER_CALL blocks each, queues
                # round-robin so all four Q7 pairs generate descriptors
                for base_tbl, nblk, nidx_half, coff in (
                    (0, ka, nidxA[t], 0),
                    (SPLIT, kb, nidxB[t], ka),
                ):
                    k0 = 0
                    while k0 < nblk:
                        k1 = min(k0 + BLKS_PER_CALL, nblk)
                        nidx = min((k1 - k0) * P, nidx_half - k0 * P)
                        gcol = (coff + k0) * E
                        icol = (b0[t] + coff + k0) * 8
                        nc.gpsimd.dma_gather(
                            out_ap=g[:, gcol : gcol + (k1 - k0) * E].rearrange(
                                "p (n e) -> p n e", e=E
                            ),
                            in_ap=feat2[base_tbl:, :],
                            idxs_ap=idx_sb[:, icol : icol + (nidx + 15) // 16],
                            num_idxs=nidx,
                            num_idxs_reg=nidx,
                            elem_size=E,
                            queue_num=qn % NQUEUES,
                        )
                        qn += 1
                        k0 = k1
                # batched strip-S build: one is_equal + one mult for all
                # nb blocks, on [128, W] strips (edges are dst-sorted, so
                # each block only spans a narrow dst band)
                s = spool.tile([P, swmax], f16, tag="s")
                nc.vector.tensor_tensor(
                    out=s[:, : nb * W].rearrange("p (n q) -> p n q", q=W),
                    in0=iota_sb[:, :W].rearrange("p (n q) -> p n q", n=1).to_broadcast(
                        [P, nb, W]
                    ),
                    in1=dstrel_sb[:, b0[t] : b0[t] + nb].to_broadcast([P, nb, W]),
                    op=mybir.AluOpType.is_equal,
                )
                nc.vector.tensor_tensor(
                    out=s[:, : nb * W].rearrange("p (n q) -> p n q", q=W),
                    in0=s[:, : nb * W].rearrange("p (n q) -> p n q", q=W),
                    in1=wv_sb[:, b0[t] : b0[t] + nb].to_broadcast([P, nb, W]),
                    op=mybir.AluOpType.mult,
                )
                nt = pnT.tile([D, P], f32)
                # zeroing matmul over the full [64, 128] region
                nc.tensor.matmul(
                    out=nt[:], lhsT=g[:, :D], rhs=zeros_sb[:], start=True, stop=False
                )
                for i in range(nb):
                    a = a_blk[b0[t] + i]
                    nc.tensor.matmul(
                        out=nt[:, a : a + W],
                        lhsT=g[:, i * E : i * E + D],
                        rhs=s[:, i * W : (i + 1) * W],
                        start=False,
                        stop=False,
                    )
                nc.tensor.matmul(
                    out=nt[:], lhsT=g[:, :D], rhs=zeros_sb[:], start=False, stop=True
                )
                comb = cpool.tile([P, P], f16, tag="comb")
                nc.sync.dma_start(out=comb[:D, :], in_=featT[:, t * P : (t + 1) * P])
                nc.scalar.activation(
                    out=comb[D:, :], in_=nt[:], func=mybir.ActivationFunctionType.Copy
                )
                po = pout.tile([P, D], f32)
                nc.tensor.matmul(
                    out=po[:], lhsT=comb[:], rhs=wt_sb[:], start=True, stop=True
                )
                o = opool.tile([P, D], f32, tag="o")
                nc.vector.tensor_add(out=o[:], in0=po[:], in1=bias_sb[:])
                sq = opool.tile([P, D], f32, tag="sq")
                ssum = stat.tile([P, 1], f32, tag="ssum")
                nc.scalar.activation(
                    out=sq[:],
                    in_=o[:],
                    func=mybir.ActivationFunctionType.Square,
                    accum_out=ssum[:],
                )
                nrm = stat.tile([P, 1], f32, tag="nrm")
                nc.scalar.activation(
                    out=nrm[:], in_=ssum[:], func=mybir.ActivationFunctionType.Sqrt
                )
                nc.vector.tensor_scalar_max(out=nrm[:], in0=nrm[:], scalar1=1e-12)
                rin = stat.tile([P, 1], f32, tag="rin")
                nc.vector.reciprocal(out=rin[:], in_=nrm[:])
                o2 = opool.tile([P, D], f32, tag="o2")
                nc.scalar.activation(
                    out=o2[:],
                    in_=o[:],
                    func=mybir.ActivationFunctionType.Copy,
                    scale=rin[:],
                )
                rows = LAST_ROWS if t == T - 1 else P
                nc.sync.dma_start(out=out[t * P : t * P + rows, :], in_=o2[:rows, :])

    nc.compile()
    return nc


def kernel(features, edge_src, edge_dst, edge_weight, W, b, _cache={}):
    global _last_results
    features = np.ascontiguousarray(features, dtype=np.float32)
    edge_src = np.ascontiguousarray(edge_src, dtype=np.int32)
    edge_dst = np.ascontiguousarray(edge_dst, dtype=np.int32)
    edge_weight = np.ascontiguousarray(edge_weight, dtype=np.float32)
    W = np.ascontiguousarray(W, dtype=np.float32)
    b = np.ascontiguousarray(b, dtype=np.float32)

    dstrel, wv, idxw, nbA, nbB, nidxA, nidxB, b0, tb, Wt, a_blk = _prep(
        edge_src, edge_dst, edge_weight
    )

    f16 = features.astype(np.float16)
    feat2 = np.ascontiguousarray(np.concatenate([f16, f16], axis=1))  # [N, 128]
    featT = features.T.astype(np.float16)
    featT_pad = np.zeros((C, D, T * P), np.float16)
    for c in range(C):
        featT_pad[c, :, :NPC] = featT[:, c * NPC : (c + 1) * NPC]
    wt = np.ascontiguousarray(W.T.astype(np.float16))
    biasb = np.ascontiguousarray(np.broadcast_to(b, (P, D)).astype(np.float32))
    iota = np.tile(np.arange(P, dtype=np.float16), (P, 1))

    key = ("k5", tb, tuple(nidxA), tuple(nidxB), tuple(Wt))
    if key not in _cache:
        _cache.clear()
        _cache[key] = _build(nbA, nbB, nidxA, nidxB, b0, tb, Wt, a_blk)
    nc = _cache[key]

    in_maps = [
        {
            "feat2": feat2,
            "idxw": np.ascontiguousarray(np.tile(idxw[c], (8, 1))),
            "dstrel": np.ascontiguousarray(dstrel[c]),
            "wv": np.ascontiguousarray(wv[c]),
            "featT": featT_pad[c],
            "wt": wt,
            "biasb": biasb,
            "iota": iota,
        }
        for c in range(C)
    ]
    import os

    trace = bool(os.environ.get("GS_TRACE"))
    res = run_bass_kernel_spmd(
        nc, in_maps, core_ids=list(range(C)), trace=trace
    )
    _last_results = res
    out = np.concatenate([res.results[c]["out"] for c in range(C)], axis=0)
    return out.astype(np.float32)
